# revision 28
# baseline (speedup 1.0000x reference)
"""DMSA (dual-modal channel cross-attention) Trainium2 kernel — v5.

Sharding: 8 cores = 2 batches x 4 bands of 32 image rows. Each core
computes its band fully; the channel attention's per-head Gram matrices
(contraction over all n = h*w tokens, with l2-normalization folded in
via the Gram diagonal) are summed with one AllReduce per 4-core group.

Key design points (vs the 606us v2 baseline, ~1.7x faster):
- All weights packed into one bf16 + one f32r blob (4 big DMAs): the
  v2 per-tensor loads produced thousands of 8-byte DMA packets that
  serialized ~100us of dead time before the first matmul.
- v kept in SBUF (bf16 [128,2,36,130] per modality); no DRAM spill or
  re-read (-30MB DMA/core), conv taps and stage-3 matmul rhs read it
  in place.
- Whole stage-1 stream in bf16 (PE forbids mixing 32/16-bit matmul
  operands); f32r retained for the tiny softmax/proj-matrix path.
- Activation-table discipline: Prelu (not Lrelu) + Copy + Gelu all
  live in one Act table set -> 4 table loads total instead of 37.
  PSUM evictions ride the Act engine (Copy) to keep DVE free for the
  9-tap conv1; conv1 output gelu'd in place per chunk.
- Compact f32 collective payload [128,8,64] (own-block diag cols +
  cross block) instead of the full [8,128,128] Gram: 256KB vs 512KB,
  extracted straight from PSUM.
- conv2 (pos-emb second dwconv) as 9 diagonal matmuls per 512-token
  block runs during the collective window into a bf16 accumulator;
  the post-collective pass is only the fused attn+proj matmul plus a
  single DVE (psum + bias + acc) combine per tile.
"""
import numpy as np
import ml_dtypes
from contextlib import ExitStack

import concourse.bass as bass
import concourse.tile as tile
import concourse.mybir as mybir
from concourse import bacc
from concourse.bass_utils import run_bass_kernel_spmd

F32 = mybir.dt.float32
F32R = mybir.dt.float32r
BF16 = mybir.dt.bfloat16
AF = mybir.ActivationFunctionType
OP = mybir.AluOpType

B, H, W, C = 2, 128, 128, 256
HEADS, DH = 8, 32
RB = 32             # image rows per core
ER = RB + 4         # ext rows
WP = W + 2          # padded width (SBUF v grid)
EN = ER * W         # unpadded ext tokens (stage-1 grid) = 4608
NV = RB * W         # valid tokens = 4096
NT = 9              # stage-1 tiles (4 ext rows each)
LRELU_A = 0.01
# conv1 chunk g-row ranges and the stage-1 tile after which each may run
C1CHUNKS = [(0, 6, 1), (6, 12, 3), (12, 18, 4), (18, 24, 6), (24, 30, 7),
            (30, 34, None)]  # None -> last (x on DVE so CC isn't delayed)

# packed-weight layouts: (name, shape-after-partition-dim)
SPEC_F32 = [
    ("pxwT", (2, 256)), ("pywT", (2, 256)),
    ("blk128", (128,)), ("eye32r", (32,)), ("eye4", (2, 2, 32)),
    ("w1c", (2, 9)),
    ("bfx", (2,)), ("bfy", (2,)), ("bq", (2,)), ("bkx", (2,)),
    ("bky", (2,)), ("bv", (2,)), ("obx", (2,)), ("oby", (2,)),
    ("b1c", (2,)), ("rx_exp", (2,)), ("ry_exp", (2,)),
    ("gm0", (1,)), ("gm33", (1,)),
]
SPEC_B16 = [
    ("fxw1T", (4, 2, 128)), ("fyw1T", (4, 2, 128)),
    ("qw1T", (2, 2, 128)), ("kxw1T", (2, 2, 128)), ("kyw1T", (2, 2, 128)),
    ("vw1T", (2, 2, 128)), ("vw2T", (2, 2, 128)),
    ("qw2T", (2, 256)), ("kw2T", (2, 256)), ("dw2", (2, 9, 128)),
]


def _spec_offsets(spec):
    offs, o = {}, 0
    for name, tail in spec:
        n = int(np.prod(tail))
        offs[name] = (o, n, tail)
        o += n
    return offs, o


OFF_F32, NF32 = _spec_offsets(SPEC_F32)
OFF_B16, NB16 = _spec_offsets(SPEC_B16)
# bf16 blob split points so the tile-0 weights land first
WB_SPLITS = [0, 2560, 5120, NB16]

_CACHED = {}


def _nc_build():
    nc = bacc.Bacc(num_devices=8)

    xin = nc.dram_tensor("xin", [128, 2, EN], BF16, kind="ExternalInput")
    yin = nc.dram_tensor("yin", [128, 2, EN], BF16, kind="ExternalInput")
    wfd = nc.dram_tensor("wf", [128, NF32], F32R, kind="ExternalInput")
    wbd = nc.dram_tensor("wb", [128, NB16], BF16, kind="ExternalInput")

    out_x = nc.dram_tensor("out_x", [128, 2, NV], F32, kind="ExternalOutput")
    out_y = nc.dram_tensor("out_y", [128, 2, NV], F32, kind="ExternalOutput")
    cc_in = nc.dram_tensor("cc_in", [128, 8, 64], F32, kind="Internal")
    cc_out = nc.dram_tensor("cc_out", [128, 8, 64], F32, kind="Internal")

    with tile.TileContext(nc) as tc, ExitStack() as ctx:
        wp = ctx.enter_context(tc.tile_pool(name="wp", bufs=1))
        io = ctx.enter_context(tc.tile_pool(name="io", bufs=3))
        hidF = ctx.enter_context(tc.tile_pool(name="hidF", bufs=2))
        hidQ = ctx.enter_context(tc.tile_pool(name="hidQ", bufs=2))
        hidV = ctx.enter_context(tc.tile_pool(name="hidV", bufs=2))
        stk = ctx.enter_context(tc.tile_pool(name="stk", bufs=2))
        sm = ctx.enter_context(tc.tile_pool(name="sm", bufs=1))
        gb = ctx.enter_context(tc.tile_pool(name="gb", bufs=1))
        vbp = ctx.enter_context(tc.tile_pool(name="vbp", bufs=1))
        accp = ctx.enter_context(tc.tile_pool(name="accp", bufs=1))
        ot = ctx.enter_context(tc.tile_pool(name="ot", bufs=2))
        psA = ctx.enter_context(tc.tile_pool(name="psA", bufs=2, space="PSUM"))
        psQ = ctx.enter_context(tc.tile_pool(name="psQ", bufs=2, space="PSUM"))
        psG = ctx.enter_context(tc.tile_pool(name="psG", bufs=1, space="PSUM"))

        # input tile 0 DMAs go first so their packets lead the queue
        xt0 = io.tile([128, 2, 512], BF16, tag="xt")
        nc.sync.dma_start(xt0[:], xin.ap()[:, :, 0:512])
        yt0 = io.tile([128, 2, 512], BF16, tag="yt")
        nc.sync.dma_start(yt0[:], yin.ap()[:, :, 0:512])

        wf = wp.tile([128, NF32], F32R, tag="wf")
        nc.sync.dma_start(wf[:], wfd.ap())
        wb = wp.tile([128, NB16], BF16, tag="wb")
        for a, b in zip(WB_SPLITS[:-1], WB_SPLITS[1:]):
            nc.sync.dma_start(wb[:, a:b], wbd.ap()[:, a:b])

        wff = wf.bitcast(F32)

        def wview(name):
            if name in OFF_F32:
                o, n, tail = OFF_F32[name]
                v = wff[:, o:o + n]
            else:
                o, n, tail = OFF_B16[name]
                v = wb[:, o:o + n]
            if len(tail) == 2:
                v = v.rearrange("p (a b) -> p a b", a=tail[0])
            elif len(tail) == 3:
                v = v.rearrange("p (a b c) -> p a b c", a=tail[0], b=tail[1])
            return v

        def wviewr(name):
            o, n, tail = OFF_F32[name]
            v = wf[:, o:o + n]
            if len(tail) == 2:
                v = v.rearrange("p (a b) -> p a b", a=tail[0])
            elif len(tail) == 3:
                v = v.rearrange("p (a b c) -> p a b c", a=tail[0], b=tail[1])
            return v

        w = {}
        for name in ("pxwT", "pywT", "blk128"):
            w[name] = wviewr(name)
        for name in ("eye32r", "eye4", "w1c", "bfx", "bfy", "bq", "bkx", "bky",
                     "bv", "obx", "oby", "b1c", "rx_exp", "ry_exp",
                     "gm0", "gm33"):
            w[name] = wview(name)
        for name, _ in SPEC_B16:
            w[name] = wview(name)

        def scopy(out, in_):
            # Copy is resident in every Act table set -> never a table load
            nc.scalar.activation(out, in_, AF.Copy)

        # SBUF v grids (bf16, width-padded); zero the pad columns once
        vbx = vbp.tile([128, 2, ER, WP], BF16, tag="vbx")
        vby = vbp.tile([128, 2, ER, WP], BF16, tag="vby")
        for vb in (vbx, vby):
            nc.vector.memset(vb[:, :, :, 0:1], 0.0)
            nc.vector.memset(vb[:, :, :, WP - 1:WP], 0.0)

        gram0 = psG.tile([128, 512], F32, tag="gram0")
        gram1 = psG.tile([128, 512], F32, tag="gram1")
        grams = [gram0, gram1]

        gx = gb.tile([128, 2, ER - 2, WP], BF16, tag="gx")
        gy = gb.tile([128, 2, ER - 2, WP], BF16, tag="gy")
        nc.scalar.memzero(gx[:])
        nc.scalar.memzero(gy[:])
        TAPS = [(dr, dc) for dr in (-1, 0, 1) for dc in (-1, 0, 1)]

        def conv1_chunk(gbuf, vb, g0, g1):
            """9-tap conv1 for g rows [g0, g1), accumulated in gbuf (bf16,
            pre-gelu; the gelu+bias pass is batched at stage-1 end)."""
            nr = g1 - g0
            for g in range(2):
                dst = gbuf[:, g, g0:g1, 1:129]
                for i, (dr, dc) in enumerate(TAPS):
                    src = vb[:, g, g0 + 1 + dr:g0 + 1 + dr + nr,
                             1 + dc:129 + dc]
                    if i == 0:
                        nc.vector.tensor_scalar_mul(dst, src,
                                                    w["w1c"][:, g, 0:1])
                    else:
                        nc.vector.scalar_tensor_tensor(
                            dst, src, w["w1c"][:, g, i:i + 1],
                            dst, OP.mult, OP.add)

        def gelu_pass(gbuf, r0, r1):
            for g in range(2):
                nc.scalar.activation(gbuf[:, g, r0:r1, 1:129],
                                     gbuf[:, g, r0:r1, 1:129],
                                     AF.Gelu, bias=w["b1c"][:, g:g + 1])

        # ================= stage 1 =================
        vrow = 0

        def mlp1(srcs, w1T, nk, bias, tag, pool, dt, lo=0, n=512):
            """hidden = lrelu(srcs @ w1T + b); paired-bank PSUM."""
            ht = pool.tile([128, 2, 512], dt, tag=tag)
            ps = psA.tile([128, 2, 512], F32, tag="psA")
            for mh in range(2):
                for k in range(nk):
                    src = srcs[k // 2][:, k % 2, lo:lo + n] if len(srcs) > 1 \
                        else srcs[0][:, k, lo:lo + n]
                    nc.tensor.matmul(ps[:, mh, :n], w1T[:, k, mh, :], src,
                                     start=(k == 0), stop=(k == nk - 1))
            for mh in range(2):
                nc.scalar.activation(ht[:, mh, :n], ps[:, mh, :n], AF.Prelu,
                                     bias=bias[:, mh:mh + 1], alpha=LRELU_A)
            return ht

        for t in range(NT):
            if t == 0:
                xt, yt = xt0, yt0
            else:
                xt = io.tile([128, 2, 512], BF16, tag="xt")
                nc.sync.dma_start(xt[:], xin.ap()[:, :, t * 512:(t + 1) * 512])
                yt = io.tile([128, 2, 512], BF16, tag="yt")
                nc.sync.dma_start(yt[:], yin.ap()[:, :, t * 512:(t + 1) * 512])

            # valid-row window within this tile
            e0, e1 = max(2, 4 * t), min(ER - 2, 4 * t + 4)
            lo, n = (e0 - 4 * t) * 128, (e1 - e0) * 128

            fhy = mlp1([xt, yt], w["fyw1T"], 4, w["bfy"], "fhy", hidF, BF16,
                       lo, n)
            fhx = mlp1([xt, yt], w["fxw1T"], 4, w["bfx"], "fhx", hidF, BF16,
                       lo, n)
            vhx = mlp1([xt], w["vw1T"], 2, w["bv"], "vhx", hidV, BF16)
            vhy = mlp1([yt], w["vw1T"], 2, w["bv"], "vhy", hidV, BF16)
            qhx = mlp1([xt], w["qw1T"], 2, w["bq"], "qhx", hidQ, BF16, lo, n)
            qhy = mlp1([yt], w["qw1T"], 2, w["bq"], "qhy", hidQ, BF16, lo, n)
            khy = mlp1([fhy], w["kyw1T"], 2, w["bky"], "khy", hidQ, BF16,
                       0, n)
            khx = mlp1([fhx], w["kxw1T"], 2, w["bkx"], "khx", hidQ, BF16,
                       0, n)

            # v = vhid @ vw2T (ext tokens), evict bf16 into the SBUF grid
            for vh, vb in ((vhx, vbx), (vhy, vby)):
                ps = psA.tile([128, 2, 512], F32, tag="psA")
                for mh in range(2):
                    for k in range(2):
                        nc.tensor.matmul(ps[:, mh, :], w["vw2T"][:, k, mh, :],
                                         vh[:, k, :], start=(k == 0),
                                         stop=(k == 1))
                scopy(vb[:, :, 4 * t:4 * t + 4, 1:129],
                      ps.rearrange("p a (r c) -> p a r c", c=128))

            # token-major QK L2 + Gram per valid image row; two stacks
            # share one PSUM tile and one eviction
            for e in range(e0, e1):
                off = (e - e0) * 128
                st = stk.tile([128, HEADS, 4, DH], BF16, tag="st")
                for pair, grp in enumerate((((khy, "kw2T"), (qhx, "qw2T")),
                                            ((khx, "kw2T"), (qhy, "qw2T")))):
                    ps = psQ.tile([128, 2, 256], F32, tag="psQ")
                    for s2, (hh, w2T) in enumerate(grp):
                        for k in range(2):
                            nc.tensor.matmul(ps[:, s2, :],
                                             hh[:, k, off:off + 128],
                                             w[w2T][:, k, :], start=(k == 0),
                                             stop=(k == 1))
                    scopy(st[:, :, 2 * pair:2 * pair + 2, :],
                          ps.rearrange("p s (h d) -> p h s d", h=HEADS))
                for h in range(HEADS):
                    nc.tensor.matmul(
                        grams[h // 4][:, (h % 4) * 128:(h % 4) * 128 + 128],
                        st[:, h], st[:, h],
                        start=(vrow == 0), stop=(vrow == RB - 1),
                        skip_group_check=True)
                vrow += 1

            # interleaved conv1 chunks with in-place gelu per chunk
            for g0, g1, after in C1CHUNKS:
                if after == t:
                    for gbuf, vb in ((gx, vbx), (gy, vby)):
                        conv1_chunk(gbuf, vb, g0, g1)
                        gelu_pass(gbuf, g0, g1)
                        if g0 == 0:
                            nc.vector.tensor_scalar_mul(
                                gbuf[:, :, 0, :], gbuf[:, :, 0, :],
                                w["gm0"][:])

        # ============ compact Gram payload -> AllReduce ============
        # csb [128(stack: ky|qx|kx|qy x32), head, 64]:
        #   cols 0:32  = own-block (diag blocks, for the l2 norms)
        #   cols 32:64 = cross block (B1 = ky^T qx at p 0:32,
        #                             B2 = kx^T qy at p 64:96)
        csb = sm.tile([128, 8, 64], F32, tag="csb")
        nc.vector.memset(csb[:], 0.0)
        for g in range(2):
            grv = grams[g].rearrange("p (h c) -> p h c", h=4)
            for pr in range(4):
                nc.vector.tensor_copy(
                    csb[pr * 32:(pr + 1) * 32, 4 * g:4 * g + 4, 0:32],
                    grv[pr * 32:(pr + 1) * 32, :, pr * 32:pr * 32 + 32])
            nc.vector.tensor_copy(csb[0:32, 4 * g:4 * g + 4, 32:64],
                                  grv[0:32, :, 32:64])
            nc.vector.tensor_copy(csb[64:96, 4 * g:4 * g + 4, 32:64],
                                  grv[64:96, :, 96:128])
        nc.sync.dma_start(cc_in.ap(), csb[:])
        nc.gpsimd.collective_compute(
            "AllReduce", OP.add,
            ins=[cc_in.ap()], outs=[cc_out.ap()],
            replica_groups=[[0, 1, 2, 3], [4, 5, 6, 7]])

        # last conv1 chunk + its gelu + boundary mask overlap the CC
        for g0, g1, after in C1CHUNKS:
            if after is None:
                for gbuf, vb in ((gx, vbx), (gy, vby)):
                    conv1_chunk(gbuf, vb, g0, g1)
                    gelu_pass(gbuf, g0, g1)
                    nc.vector.tensor_scalar_mul(
                        gbuf[:, :, ER - 3, :], gbuf[:, :, ER - 3, :],
                        w["gm33"][:])


        # ====== conv2 (pos-emb second dwconv) during the collective ======
        # 9 diagonal matmuls per 512-token block into PSUM, evicted to a
        # bf16 accumulator; the post-collective pass only adds the proj.
        acc_x = accp.tile([128, 2, RB, 128], BF16, tag="acc_x")
        acc_y = accp.tile([128, 2, RB, 128], BF16, tag="acc_y")
        accs = {"x": acc_x, "y": acc_y}

        def conv2_block(gbuf, acc, tt, on_dve):
            ps = psA.tile([128, 2, 512], F32, tag="psA")
            for mo in range(2):
                for i in range(9):
                    dr, dc = TAPS[i]
                    src = gbuf[:, mo, 4 * tt + 1 + dr:4 * tt + 5 + dr,
                               1 + dc:129 + dc]
                    nc.tensor.matmul(ps[:, mo, :], w["dw2"][:, mo, i, :],
                                     src, start=(i == 0), stop=(i == 8),
                                     skip_group_check=True)
            dst = acc[:, :, 4 * tt:4 * tt + 4, :]
            srcv = ps.rearrange("p a (r c) -> p a r c", c=128)
            if on_dve:
                nc.vector.tensor_copy(dst, srcv)
            else:
                scopy(dst, srcv)

        for tt in range(7):
            conv2_block(gx, acc_x, tt, False)
            if tt < 4:
                conv2_block(gy, acc_y, tt, False)
        conv2_block(gx, acc_x, 7, False)

        # ========== softmax + BD + fused proj matrices ==========
        # layouts from cc_out [128(stack), 8, 64] f32:
        #   x: cross at p 0:32, own k at p 0:32, own q at p 32:64
        #   y: cross at p 64:96, own k at p 64:96, own q at p 96:128
        PRE = {"x": (0, 0), "y": (64, 64)}
        s_ts, dbs = {}, {}
        for d, (pc, pk) in PRE.items():
            s_t = sm.tile([128, 2, DH], F32, tag=f"s_t{d}")
            nc.sync.dma_start(
                s_t[:],
                cc_out.ap()[pc:pc + 32, :, 32:64]
                .rearrange("d (g j) e -> j d g e", g=2))
            db = sm.tile([128, 2, 2, DH], F32, tag=f"db{d}")
            for jj in range(2):
                nc.sync.dma_start(
                    db[:, :, jj, :],
                    cc_out.ap()[pk + 32 * jj:pk + 32 * jj + 32, :, 0:32]
                    .rearrange("d (g j) e -> j d g e", g=2))
            s_ts[d], dbs[d] = s_t, db

        def softmax_m1t(d):
            rexp = "rx_exp" if d == "x" else "ry_exp"
            pwT = "pxwT" if d == "x" else "pywT"
            s_t, db = s_ts[d], dbs[d]
            dbf = sm.tile([128, 2, 2, DH], F32, tag="dbf")
            nc.vector.tensor_tensor(dbf[:], db[:], w["eye4"][:], OP.mult)
            nkq = sm.tile([128, 2, 2], F32, tag="nkq")
            nc.vector.tensor_reduce(nkq[:], dbf[:], mybir.AxisListType.X,
                                    OP.add)
            inv = sm.tile([128, 2, 2], F32, tag="inv")
            nc.scalar.sqrt(inv[:], nkq[:])
            nc.vector.tensor_scalar_max(inv[:], inv[:], 1e-12)
            nc.vector.reciprocal(inv[:], inv[:])
            ks = sm.tile([128, 2], F32, tag="ks")
            nc.vector.tensor_tensor(ks[:], inv[:, :, 0], w[rexp][:], OP.mult)
            qs = sm.tile([128, 2, DH], F32, tag="qs")
            for g in range(2):
                eis = sm.tile([128, DH], F32, tag="eis")
                nc.vector.tensor_scalar_mul(eis[:], w["eye32r"][:],
                                            inv[:, g, 1:2])
                ei = sm.tile([128, DH], F32R, tag="ei")
                nc.vector.tensor_copy(ei[:], eis[:])
                pq_ = psQ.tile([128, DH], F32, tag="psQ")
                nc.tensor.matmul(pq_[:], w["blk128"][:], ei[:],
                                 start=True, stop=True)
                scopy(qs[:, g, :], pq_[:])
            # logits are cosine similarities (|lg| <= rescale), so exp is
            # safe without the max-subtraction pass
            lg = sm.tile([128, 2, DH], F32, tag="lg")
            for g in range(2):
                nc.vector.scalar_tensor_tensor(lg[:, g, :], s_t[:, g, :],
                                               ks[:, g:g + 1], qs[:, g, :],
                                               OP.mult, OP.mult)
            pe_ = sm.tile([128, 2, DH], F32, tag="pe_")
            ssum = sm.tile([128, 2], F32, tag="ssum")
            for g in range(2):
                nc.scalar.activation(pe_[:, g, :], lg[:, g, :], AF.Exp,
                                     accum_out=ssum[:, g:g + 1])
            nc.vector.reciprocal(ssum[:], ssum[:])
            at = sm.tile([128, 2, DH], F32, tag="at")
            for g in range(2):
                nc.vector.tensor_scalar_mul(at[:, g, :], pe_[:, g, :],
                                            ssum[:, g:g + 1])
            bds = sm.tile([128, 2, 256], F32R, tag="bds")
            nc.vector.memset(bds.bitcast(F32)[:], 0.0)
            for g in range(2):
                for j in range(4):
                    h = 4 * g + j
                    nc.vector.tensor_copy(
                        bds[j * DH:(j + 1) * DH, g, h * DH:(h + 1) * DH],
                        at[j * DH:(j + 1) * DH, g, :])
            m1t = sm.tile([128, 2, 2, 128], BF16, tag=f"m1t_{d}")
            for me in range(2):
                ps = psQ.tile([128, 256], F32, tag="psQ")
                for g in range(2):
                    nc.tensor.matmul(ps[:],
                                     bds[:, g, me * 128:me * 128 + 128],
                                     w[pwT][:, g, :], start=(g == 0),
                                     stop=(g == 1))
                scopy(m1t[:, me, :, :],
                      ps.rearrange("p (a b) -> p a b", a=2))
            return m1t

        # ========== final: proj in PSUM, + bias + conv2-acc, store ==========
        def proj_pass(d, m1t):
            vb, ob, o_dram, acc = {
                "x": (vbx, "obx", out_x, acc_x),
                "y": (vby, "oby", out_y, acc_y),
            }[d]
            for tt in range(8):
                ps = psA.tile([128, 2, 512], F32, tag="psA")
                for mo in range(2):
                    for ke in range(2):
                        rhs = vb[:, ke, 4 * tt + 2:4 * tt + 6, 1:129]
                        nc.tensor.matmul(ps[:, mo, :], m1t[:, ke, mo, :], rhs,
                                         start=(ke == 0), stop=(ke == 1),
                                         skip_group_check=True)
                o_t = ot.tile([128, 2, 4, 128], F32, tag="o_t")
                for mo in range(2):
                    nc.vector.scalar_tensor_tensor(
                        o_t[:, mo, :, :],
                        ps[:, mo, :].rearrange("p (r c) -> p r c", c=128),
                        w[ob][:, mo:mo + 1],
                        acc[:, mo, 4 * tt:4 * tt + 4, :],
                        OP.add, OP.add)
                nc.sync.dma_start(
                    o_dram.ap()[:, :, tt * 512:(tt + 1) * 512],
                    o_t.rearrange("p a r c -> p a (r c)"))

        m1t_x = softmax_m1t("x")
        # the remaining conv2 blocks give the PE work to chew on while
        # the x softmax chain runs on Act/DVE
        for tt in range(4, 8):
            conv2_block(gy, acc_y, tt, False)
        m1t_y = softmax_m1t("y")
        proj_pass("x", m1t_x)
        proj_pass("y", m1t_y)

    nc.finalize()
    return nc


# ======================= host side =======================

def _prep_core_input(full, b, h0):
    """(H, W, C) rows [h0-2, h0+34) -> channel-major [128, 2, EN] f32
    (zeros outside the image)."""
    arr = np.zeros((ER, W, C), np.float32)
    r0, r1 = h0 - 2, h0 + RB + 2
    cr0, cr1 = max(r0, 0), min(r1, H)
    arr[cr0 - r0:cr1 - r0] = full[b, cr0:cr1]
    cm = arr.transpose(2, 0, 1).reshape(2, 128, EN)
    return np.ascontiguousarray(cm.transpose(1, 0, 2)).astype(ml_dtypes.bfloat16)


def _cm(v):
    return np.ascontiguousarray(v.reshape(2, 128).T.astype(np.float32))


def _lhsT(wm, nk):
    t = wm.T.reshape(nk, 128, 2, 128)
    return np.ascontiguousarray(t.transpose(1, 0, 2, 3).astype(np.float32))


def _rhsT(wm, dt=np.float32):
    t = wm.T.reshape(2, 128, wm.shape[0])
    return np.ascontiguousarray(t.transpose(1, 0, 2).astype(dt))


def _pack(parts, spec, offs, total, dtype):
    blob = np.zeros((128, total), dtype)
    for name, _ in spec:
        o, n, tail = offs[name]
        blob[:, o:o + n] = parts[name].reshape(128, n).astype(dtype)
    return blob


def kernel(_trace=False, **inputs):
    inp = {k: np.asarray(v) for k, v in inputs.items()}
    bf = ml_dtypes.bfloat16

    w2c = inp["pe_w2"].reshape(256, 9).astype(np.float32)
    dw2 = np.zeros((128, 2, 9, 128), np.float32)
    for g in range(2):
        for t in range(9):
            dw2[np.arange(128), g, t, np.arange(128)] = \
                w2c[g * 128:(g + 1) * 128, t]

    pf = {
        "pxwT": _rhsT(inp["px_w"]), "pywT": _rhsT(inp["py_w"]),
        "blk128": np.kron(np.eye(4), np.ones((32, 32))).astype(np.float32),
        "eye32r": np.tile(np.eye(32), (4, 1)).astype(np.float32),
        "eye4": np.ascontiguousarray(np.broadcast_to(
            np.tile(np.eye(32), (4, 1))[:, None, None, :],
            (128, 2, 2, 32))).astype(np.float32),
        "w1c": np.ascontiguousarray(
            inp["pe_w1"].reshape(256, 9).reshape(2, 128, 9)
            .transpose(1, 0, 2).astype(np.float32)),
        "bfx": _cm(inp["fx_b1"]), "bfy": _cm(inp["fy_b1"]),
        "bq": _cm(inp["q_b1"]), "bv": _cm(inp["v_b1"]),
        "bkx": _cm(inp["k_w1"] @ inp["fx_b2"] + inp["k_b1"]),
        "bky": _cm(inp["k_w1"] @ inp["fy_b2"] + inp["k_b1"]),
        "obx": _cm(inp["px_b"] + inp["pe_b2"]),
        "oby": _cm(inp["py_b"] + inp["pe_b2"]),
        "b1c": _cm(inp["pe_b1"]),
        "rx_exp": np.ascontiguousarray(
            np.repeat(inp["rescale_x"].reshape(2, 4), 32, axis=1).T
            .astype(np.float32)),
        "ry_exp": np.ascontiguousarray(
            np.repeat(inp["rescale_y"].reshape(2, 4), 32, axis=1).T
            .astype(np.float32)),
        "gm0": np.ones((128, 1), np.float32),
        "gm33": np.ones((128, 1), np.float32),
    }
    pb = {
        "fxw1T": _lhsT(inp["fx_w1"], 4), "fyw1T": _lhsT(inp["fy_w1"], 4),
        "qw1T": _lhsT(inp["q_w1"], 2), "vw1T": _lhsT(inp["v_w1"], 2),
        "kxw1T": _lhsT(inp["k_w1"] @ inp["fx_w2"], 2),
        "kyw1T": _lhsT(inp["k_w1"] @ inp["fy_w2"], 2),
        "vw2T": _lhsT(inp["v_w2"], 2),
        "qw2T": _rhsT(inp["q_w2"], bf), "kw2T": _rhsT(inp["k_w2"], bf),
        "dw2": dw2.astype(bf),
    }
    wf_shared = _pack(pf, SPEC_F32, OFF_F32, NF32, np.float32)
    wb_shared = _pack(pb, SPEC_B16, OFF_B16, NB16, bf)
    o0 = OFF_F32["gm0"][0]
    o33 = OFF_F32["gm33"][0]

    in_maps = []
    for r in range(8):
        b, h0 = r // 4, (r % 4) * RB
        wf = wf_shared.copy()
        wf[:, o0] = 0.0 if h0 == 0 else 1.0
        wf[:, o33] = 0.0 if h0 + RB == H else 1.0
        in_maps.append({
            "xin": _prep_core_input(inp["x_in"], b, h0),
            "yin": _prep_core_input(inp["y_in"], b, h0),
            "wf": wf,
            "wb": wb_shared,
        })

    if "nc" not in _CACHED:
        _CACHED["nc"] = _nc_build()
    res = run_bass_kernel_spmd(_CACHED["nc"], in_maps,
                               core_ids=list(range(8)), trace=_trace)
    _CACHED["last_result"] = res

    out_x = np.empty((B, H, W, C), np.float32)
    out_y = np.empty((B, H, W, C), np.float32)
    for r in range(8):
        b, h0 = r // 4, (r % 4) * RB
        for name, dst in (("out_x", out_x), ("out_y", out_y)):
            a = res.results[r][name].reshape(128, 2, RB, W)
            dst[b, h0:h0 + RB] = a.transpose(2, 3, 1, 0).reshape(RB, W, C)
    return out_x, out_y


# revision 29
# speedup vs baseline: 1.0785x; 1.0785x over previous
"""DMSA (dual-modal channel cross-attention) Trainium2 kernel — v5.

Sharding: 8 cores = 2 batches x 4 bands of 32 image rows. Each core
computes its band fully; the channel attention's per-head Gram matrices
(contraction over all n = h*w tokens, with l2-normalization folded in
via the Gram diagonal) are summed with one AllReduce per 4-core group.

Key design points (vs the 606us v2 baseline, ~1.7x faster):
- All weights packed into one bf16 + one f32r blob (4 big DMAs): the
  v2 per-tensor loads produced thousands of 8-byte DMA packets that
  serialized ~100us of dead time before the first matmul.
- v kept in SBUF (bf16 [128,2,36,130] per modality); no DRAM spill or
  re-read (-30MB DMA/core), conv taps and stage-3 matmul rhs read it
  in place.
- Whole stage-1 stream in bf16 (PE forbids mixing 32/16-bit matmul
  operands); f32r retained for the tiny softmax/proj-matrix path.
- Activation-table discipline: Prelu (not Lrelu) + Copy + Gelu all
  live in one Act table set -> 4 table loads total instead of 37.
  PSUM evictions ride the Act engine (Copy) to keep DVE free for the
  9-tap conv1; conv1 output gelu'd in place per chunk.
- Compact f32 collective payload [128,8,64] (own-block diag cols +
  cross block) instead of the full [8,128,128] Gram: 256KB vs 512KB,
  extracted straight from PSUM.
- conv2 (pos-emb second dwconv) as 9 diagonal matmuls per 512-token
  block runs during the collective window into a bf16 accumulator;
  the post-collective pass is only the fused attn+proj matmul plus a
  single DVE (psum + bias + acc) combine per tile.
"""
import numpy as np
import ml_dtypes
from contextlib import ExitStack

import concourse.bass as bass
import concourse.tile as tile
import concourse.mybir as mybir
from concourse import bacc
from concourse.bass_utils import run_bass_kernel_spmd

F32 = mybir.dt.float32
F32R = mybir.dt.float32r
BF16 = mybir.dt.bfloat16
AF = mybir.ActivationFunctionType
OP = mybir.AluOpType

B, H, W, C = 2, 128, 128, 256
HEADS, DH = 8, 32
RB = 32             # image rows per core
ER = RB + 4         # ext rows
WP = W + 2          # padded width (SBUF v grid)
EN = ER * W         # unpadded ext tokens (stage-1 grid) = 4608
NV = RB * W         # valid tokens = 4096
NT = 9              # stage-1 tiles (4 ext rows each)
LRELU_A = 0.01
# conv1 chunk g-row ranges and the stage-1 tile after which each may run
C1CHUNKS = [(0, 6, 1), (6, 12, 3), (12, 18, 4), (18, 24, 6), (24, 30, 7),
            (30, 34, None)]  # None -> last (x on DVE so CC isn't delayed)

# packed-weight layouts: (name, shape-after-partition-dim)
SPEC_F32 = [
    ("pxwT", (2, 256)), ("pywT", (2, 256)),
    ("blk128", (128,)), ("eye32r", (32,)), ("eye4", (2, 2, 32)),
    ("w1c", (2, 9)),
    ("bfx", (2,)), ("bfy", (2,)), ("bq", (2,)), ("bkx", (2,)),
    ("bky", (2,)), ("bv", (2,)), ("obx", (2,)), ("oby", (2,)),
    ("b1c", (2,)), ("rx_exp", (2,)), ("ry_exp", (2,)),
    ("gm0", (1,)), ("gm33", (1,)),
]
SPEC_B16 = [
    ("fxw1T", (4, 2, 128)), ("fyw1T", (4, 2, 128)),
    ("qw1T", (2, 2, 128)), ("kxw1T", (2, 2, 128)), ("kyw1T", (2, 2, 128)),
    ("vw1T", (2, 2, 128)), ("vw2T", (2, 2, 128)),
    ("qw2T", (2, 256)), ("kw2T", (2, 256)), ("dw2", (2, 9, 128)),
]


def _spec_offsets(spec):
    offs, o = {}, 0
    for name, tail in spec:
        n = int(np.prod(tail))
        offs[name] = (o, n, tail)
        o += n
    return offs, o


OFF_F32, NF32 = _spec_offsets(SPEC_F32)
OFF_B16, NB16 = _spec_offsets(SPEC_B16)
# bf16 blob split points so the tile-0 weights land first
WB_SPLITS = [0, 2560, 5120, NB16]

_CACHED = {}


def _nc_build():
    nc = bacc.Bacc(num_devices=8)

    xin = nc.dram_tensor("xin", [128, 2, EN], BF16, kind="ExternalInput")
    yin = nc.dram_tensor("yin", [128, 2, EN], BF16, kind="ExternalInput")
    wfd = nc.dram_tensor("wf", [128, NF32], F32R, kind="ExternalInput")
    wbd = nc.dram_tensor("wb", [128, NB16], BF16, kind="ExternalInput")

    out_x = nc.dram_tensor("out_x", [128, 2, NV], F32, kind="ExternalOutput")
    out_y = nc.dram_tensor("out_y", [128, 2, NV], F32, kind="ExternalOutput")
    cc_in = nc.dram_tensor("cc_in", [128, 8, 64], F32, kind="Internal")
    cc_out = nc.dram_tensor("cc_out", [128, 8, 64], F32, kind="Internal")

    with tile.TileContext(nc) as tc, ExitStack() as ctx:
        wp = ctx.enter_context(tc.tile_pool(name="wp", bufs=1))
        io = ctx.enter_context(tc.tile_pool(name="io", bufs=3))
        hidF = ctx.enter_context(tc.tile_pool(name="hidF", bufs=2))
        hidQ = ctx.enter_context(tc.tile_pool(name="hidQ", bufs=2))
        hidV = ctx.enter_context(tc.tile_pool(name="hidV", bufs=2))
        stk = ctx.enter_context(tc.tile_pool(name="stk", bufs=2))
        sm = ctx.enter_context(tc.tile_pool(name="sm", bufs=1))
        gb = ctx.enter_context(tc.tile_pool(name="gb", bufs=1))
        vbp = ctx.enter_context(tc.tile_pool(name="vbp", bufs=1))
        accp = ctx.enter_context(tc.tile_pool(name="accp", bufs=1))
        ot = ctx.enter_context(tc.tile_pool(name="ot", bufs=2))
        psA = ctx.enter_context(tc.tile_pool(name="psA", bufs=4, space="PSUM"))
        psQ = ctx.enter_context(tc.tile_pool(name="psQ", bufs=2, space="PSUM"))
        psG = ctx.enter_context(tc.tile_pool(name="psG", bufs=1, space="PSUM"))

        # input tile 0 DMAs go first so their packets lead the queue
        xt0 = io.tile([128, 2, 512], BF16, tag="xt")
        nc.sync.dma_start(xt0[:], xin.ap()[:, :, 0:512])
        yt0 = io.tile([128, 2, 512], BF16, tag="yt")
        nc.sync.dma_start(yt0[:], yin.ap()[:, :, 0:512])

        wf = wp.tile([128, NF32], F32R, tag="wf")
        nc.sync.dma_start(wf[:], wfd.ap())
        wb = wp.tile([128, NB16], BF16, tag="wb")
        for a, b in zip(WB_SPLITS[:-1], WB_SPLITS[1:]):
            nc.sync.dma_start(wb[:, a:b], wbd.ap()[:, a:b])

        wff = wf.bitcast(F32)

        def wview(name):
            if name in OFF_F32:
                o, n, tail = OFF_F32[name]
                v = wff[:, o:o + n]
            else:
                o, n, tail = OFF_B16[name]
                v = wb[:, o:o + n]
            if len(tail) == 2:
                v = v.rearrange("p (a b) -> p a b", a=tail[0])
            elif len(tail) == 3:
                v = v.rearrange("p (a b c) -> p a b c", a=tail[0], b=tail[1])
            return v

        def wviewr(name):
            o, n, tail = OFF_F32[name]
            v = wf[:, o:o + n]
            if len(tail) == 2:
                v = v.rearrange("p (a b) -> p a b", a=tail[0])
            elif len(tail) == 3:
                v = v.rearrange("p (a b c) -> p a b c", a=tail[0], b=tail[1])
            return v

        w = {}
        for name in ("pxwT", "pywT", "blk128"):
            w[name] = wviewr(name)
        for name in ("eye32r", "eye4", "w1c", "bfx", "bfy", "bq", "bkx", "bky",
                     "bv", "obx", "oby", "b1c", "rx_exp", "ry_exp",
                     "gm0", "gm33"):
            w[name] = wview(name)
        for name, _ in SPEC_B16:
            w[name] = wview(name)

        def scopy(out, in_):
            # Copy is resident in every Act table set -> never a table load
            nc.scalar.activation(out, in_, AF.Copy)

        # SBUF v grids (bf16, width-padded); zero the pad columns once
        vbx = vbp.tile([128, 2, ER, WP], BF16, tag="vbx")
        vby = vbp.tile([128, 2, ER, WP], BF16, tag="vby")
        for vb in (vbx, vby):
            nc.vector.memset(vb[:, :, :, 0:1], 0.0)
            nc.vector.memset(vb[:, :, :, WP - 1:WP], 0.0)

        gram0 = psG.tile([128, 512], F32, tag="gram0")
        gram1 = psG.tile([128, 512], F32, tag="gram1")
        grams = [gram0, gram1]

        gx = gb.tile([128, 2, ER - 2, WP], BF16, tag="gx")
        gy = gb.tile([128, 2, ER - 2, WP], BF16, tag="gy")
        nc.scalar.memzero(gx[:])
        nc.scalar.memzero(gy[:])
        TAPS = [(dr, dc) for dr in (-1, 0, 1) for dc in (-1, 0, 1)]

        def conv1_chunk(gbuf, vb, g0, g1):
            """9-tap conv1 for g rows [g0, g1), accumulated in gbuf (bf16,
            pre-gelu; the gelu+bias pass is batched at stage-1 end)."""
            nr = g1 - g0
            for g in range(2):
                dst = gbuf[:, g, g0:g1, 1:129]
                for i, (dr, dc) in enumerate(TAPS):
                    src = vb[:, g, g0 + 1 + dr:g0 + 1 + dr + nr,
                             1 + dc:129 + dc]
                    if i == 0:
                        nc.vector.tensor_scalar_mul(dst, src,
                                                    w["w1c"][:, g, 0:1])
                    else:
                        nc.vector.scalar_tensor_tensor(
                            dst, src, w["w1c"][:, g, i:i + 1],
                            dst, OP.mult, OP.add)

        def gelu_pass(gbuf, r0, r1):
            for g in range(2):
                nc.scalar.activation(gbuf[:, g, r0:r1, 1:129],
                                     gbuf[:, g, r0:r1, 1:129],
                                     AF.Gelu, bias=w["b1c"][:, g:g + 1])

        # ================= stage 1 =================
        vrow = 0

        def mlp1(srcs, w1T, nk, bias, tag, pool, dt, lo=0, n=512):
            """hidden = lrelu(srcs @ w1T + b); paired-bank PSUM."""
            ht = pool.tile([128, 2, 512], dt, tag=tag)
            for mh in range(2):
                ps = psA.tile([128, 512], F32, tag="psA")
                for k in range(nk):
                    src = srcs[k // 2][:, k % 2, lo:lo + n] if len(srcs) > 1 \
                        else srcs[0][:, k, lo:lo + n]
                    nc.tensor.matmul(ps[:, :n], w1T[:, k, mh, :], src,
                                     start=(k == 0), stop=(k == nk - 1))
                nc.scalar.activation(ht[:, mh, :n], ps[:, :n], AF.Prelu,
                                     bias=bias[:, mh:mh + 1], alpha=LRELU_A)
            return ht

        for t in range(NT):
            if t == 0:
                xt, yt = xt0, yt0
            else:
                xt = io.tile([128, 2, 512], BF16, tag="xt")
                nc.sync.dma_start(xt[:], xin.ap()[:, :, t * 512:(t + 1) * 512])
                yt = io.tile([128, 2, 512], BF16, tag="yt")
                nc.sync.dma_start(yt[:], yin.ap()[:, :, t * 512:(t + 1) * 512])

            # valid-row window within this tile
            e0, e1 = max(2, 4 * t), min(ER - 2, 4 * t + 4)
            lo, n = (e0 - 4 * t) * 128, (e1 - e0) * 128

            fhy = mlp1([xt, yt], w["fyw1T"], 4, w["bfy"], "fhy", hidF, BF16,
                       lo, n)
            fhx = mlp1([xt, yt], w["fxw1T"], 4, w["bfx"], "fhx", hidF, BF16,
                       lo, n)
            vhx = mlp1([xt], w["vw1T"], 2, w["bv"], "vhx", hidV, BF16)
            vhy = mlp1([yt], w["vw1T"], 2, w["bv"], "vhy", hidV, BF16)
            qhx = mlp1([xt], w["qw1T"], 2, w["bq"], "qhx", hidQ, BF16, lo, n)
            qhy = mlp1([yt], w["qw1T"], 2, w["bq"], "qhy", hidQ, BF16, lo, n)
            khy = mlp1([fhy], w["kyw1T"], 2, w["bky"], "khy", hidQ, BF16,
                       0, n)
            khx = mlp1([fhx], w["kxw1T"], 2, w["bkx"], "khx", hidQ, BF16,
                       0, n)

            # v = vhid @ vw2T (ext tokens), evict bf16 into the SBUF grid
            for vh, vb in ((vhx, vbx), (vhy, vby)):
                for mh in range(2):
                    ps = psA.tile([128, 512], F32, tag="psA")
                    for k in range(2):
                        nc.tensor.matmul(ps[:], w["vw2T"][:, k, mh, :],
                                         vh[:, k, :], start=(k == 0),
                                         stop=(k == 1))
                    scopy(vb[:, mh, 4 * t:4 * t + 4, 1:129],
                          ps.rearrange("p (r c) -> p r c", c=128))

            # token-major QK L2 + Gram per valid image row; two stacks
            # share one PSUM tile and one eviction
            for e in range(e0, e1):
                off = (e - e0) * 128
                st = stk.tile([128, HEADS, 4, DH], BF16, tag="st")
                for pair, grp in enumerate((((khy, "kw2T"), (qhx, "qw2T")),
                                            ((khx, "kw2T"), (qhy, "qw2T")))):
                    ps = psQ.tile([128, 2, 256], F32, tag="psQ")
                    for s2, (hh, w2T) in enumerate(grp):
                        for k in range(2):
                            nc.tensor.matmul(ps[:, s2, :],
                                             hh[:, k, off:off + 128],
                                             w[w2T][:, k, :], start=(k == 0),
                                             stop=(k == 1))
                    scopy(st[:, :, 2 * pair:2 * pair + 2, :],
                          ps.rearrange("p s (h d) -> p h s d", h=HEADS))
                for h in range(HEADS):
                    nc.tensor.matmul(
                        grams[h // 4][:, (h % 4) * 128:(h % 4) * 128 + 128],
                        st[:, h], st[:, h],
                        start=(vrow == 0), stop=(vrow == RB - 1),
                        skip_group_check=True)
                vrow += 1

            # interleaved conv1 chunks with in-place gelu per chunk
            for g0, g1, after in C1CHUNKS:
                if after == t:
                    for gbuf, vb in ((gx, vbx), (gy, vby)):
                        conv1_chunk(gbuf, vb, g0, g1)
                        gelu_pass(gbuf, g0, g1)
                        if g0 == 0:
                            nc.vector.tensor_scalar_mul(
                                gbuf[:, :, 0, :], gbuf[:, :, 0, :],
                                w["gm0"][:])

        # ============ compact Gram payload -> AllReduce ============
        # csb [128(stack: ky|qx|kx|qy x32), head, 64]:
        #   cols 0:32  = own-block (diag blocks, for the l2 norms)
        #   cols 32:64 = cross block (B1 = ky^T qx at p 0:32,
        #                             B2 = kx^T qy at p 64:96)
        csb = sm.tile([128, 8, 64], F32, tag="csb")
        nc.vector.memset(csb[:], 0.0)
        for g in range(2):
            grv = grams[g].rearrange("p (h c) -> p h c", h=4)
            for pr in range(4):
                nc.vector.tensor_copy(
                    csb[pr * 32:(pr + 1) * 32, 4 * g:4 * g + 4, 0:32],
                    grv[pr * 32:(pr + 1) * 32, :, pr * 32:pr * 32 + 32])
            nc.vector.tensor_copy(csb[0:32, 4 * g:4 * g + 4, 32:64],
                                  grv[0:32, :, 32:64])
            nc.vector.tensor_copy(csb[64:96, 4 * g:4 * g + 4, 32:64],
                                  grv[64:96, :, 96:128])
        nc.sync.dma_start(cc_in.ap(), csb[:])
        nc.gpsimd.collective_compute(
            "AllReduce", OP.add,
            ins=[cc_in.ap()], outs=[cc_out.ap()],
            replica_groups=[[0, 1, 2, 3], [4, 5, 6, 7]])

        # last conv1 chunk + its gelu + boundary mask overlap the CC
        for g0, g1, after in C1CHUNKS:
            if after is None:
                for gbuf, vb in ((gx, vbx), (gy, vby)):
                    conv1_chunk(gbuf, vb, g0, g1)
                    gelu_pass(gbuf, g0, g1)
                    nc.vector.tensor_scalar_mul(
                        gbuf[:, :, ER - 3, :], gbuf[:, :, ER - 3, :],
                        w["gm33"][:])


        # ====== conv2 (pos-emb second dwconv) during the collective ======
        # 9 diagonal matmuls per 512-token block into PSUM, evicted to a
        # bf16 accumulator; the post-collective pass only adds the proj.
        acc_x = accp.tile([128, 2, RB, 128], BF16, tag="acc_x")
        acc_y = accp.tile([128, 2, RB, 128], BF16, tag="acc_y")
        accs = {"x": acc_x, "y": acc_y}

        def conv2_block(gbuf, acc, tt, on_dve):
            for mo in range(2):
                ps = psA.tile([128, 512], F32, tag="psA")
                for i in range(9):
                    dr, dc = TAPS[i]
                    src = gbuf[:, mo, 4 * tt + 1 + dr:4 * tt + 5 + dr,
                               1 + dc:129 + dc]
                    nc.tensor.matmul(ps[:], w["dw2"][:, mo, i, :],
                                     src, start=(i == 0), stop=(i == 8),
                                     skip_group_check=True)
                dst = acc[:, mo, 4 * tt:4 * tt + 4, :]
                srcv = ps.rearrange("p (r c) -> p r c", c=128)
                if on_dve:
                    nc.vector.tensor_copy(dst, srcv)
                else:
                    scopy(dst, srcv)

        for tt in range(7):
            conv2_block(gx, acc_x, tt, False)
            if tt < 4:
                conv2_block(gy, acc_y, tt, False)
        conv2_block(gx, acc_x, 7, False)

        # ========== softmax + BD + fused proj matrices ==========
        # layouts from cc_out [128(stack), 8, 64] f32:
        #   x: cross at p 0:32, own k at p 0:32, own q at p 32:64
        #   y: cross at p 64:96, own k at p 64:96, own q at p 96:128
        PRE = {"x": (0, 0), "y": (64, 64)}
        s_ts, dbs = {}, {}
        for d, (pc, pk) in PRE.items():
            s_t = sm.tile([128, 2, DH], F32, tag=f"s_t{d}")
            nc.sync.dma_start(
                s_t[:],
                cc_out.ap()[pc:pc + 32, :, 32:64]
                .rearrange("d (g j) e -> j d g e", g=2))
            db = sm.tile([128, 2, 2, DH], F32, tag=f"db{d}")
            for jj in range(2):
                nc.sync.dma_start(
                    db[:, :, jj, :],
                    cc_out.ap()[pk + 32 * jj:pk + 32 * jj + 32, :, 0:32]
                    .rearrange("d (g j) e -> j d g e", g=2))
            s_ts[d], dbs[d] = s_t, db

        def softmax_m1t(d):
            rexp = "rx_exp" if d == "x" else "ry_exp"
            pwT = "pxwT" if d == "x" else "pywT"
            s_t, db = s_ts[d], dbs[d]
            dbf = sm.tile([128, 2, 2, DH], F32, tag="dbf")
            nc.vector.tensor_tensor(dbf[:], db[:], w["eye4"][:], OP.mult)
            nkq = sm.tile([128, 2, 2], F32, tag="nkq")
            nc.vector.tensor_reduce(nkq[:], dbf[:], mybir.AxisListType.X,
                                    OP.add)
            inv = sm.tile([128, 2, 2], F32, tag="inv")
            nc.scalar.sqrt(inv[:], nkq[:])
            nc.vector.tensor_scalar_max(inv[:], inv[:], 1e-12)
            nc.vector.reciprocal(inv[:], inv[:])
            ks = sm.tile([128, 2], F32, tag="ks")
            nc.vector.tensor_tensor(ks[:], inv[:, :, 0], w[rexp][:], OP.mult)
            qs = sm.tile([128, 2, DH], F32, tag="qs")
            for g in range(2):
                eis = sm.tile([128, DH], F32, tag="eis")
                nc.vector.tensor_scalar_mul(eis[:], w["eye32r"][:],
                                            inv[:, g, 1:2])
                ei = sm.tile([128, DH], F32R, tag="ei")
                nc.vector.tensor_copy(ei[:], eis[:])
                pq_ = psQ.tile([128, DH], F32, tag="psQ")
                nc.tensor.matmul(pq_[:], w["blk128"][:], ei[:],
                                 start=True, stop=True)
                scopy(qs[:, g, :], pq_[:])
            # logits are cosine similarities (|lg| <= rescale), so exp is
            # safe without the max-subtraction pass
            lg = sm.tile([128, 2, DH], F32, tag="lg")
            for g in range(2):
                nc.vector.scalar_tensor_tensor(lg[:, g, :], s_t[:, g, :],
                                               ks[:, g:g + 1], qs[:, g, :],
                                               OP.mult, OP.mult)
            pe_ = sm.tile([128, 2, DH], F32, tag="pe_")
            ssum = sm.tile([128, 2], F32, tag="ssum")
            for g in range(2):
                nc.scalar.activation(pe_[:, g, :], lg[:, g, :], AF.Exp,
                                     accum_out=ssum[:, g:g + 1])
            nc.vector.reciprocal(ssum[:], ssum[:])
            at = sm.tile([128, 2, DH], F32, tag="at")
            for g in range(2):
                nc.vector.tensor_scalar_mul(at[:, g, :], pe_[:, g, :],
                                            ssum[:, g:g + 1])
            bds = sm.tile([128, 2, 256], F32R, tag="bds")
            nc.vector.memset(bds.bitcast(F32)[:], 0.0)
            for g in range(2):
                for j in range(4):
                    h = 4 * g + j
                    nc.vector.tensor_copy(
                        bds[j * DH:(j + 1) * DH, g, h * DH:(h + 1) * DH],
                        at[j * DH:(j + 1) * DH, g, :])
            m1t = sm.tile([128, 2, 2, 128], BF16, tag=f"m1t_{d}")
            for me in range(2):
                ps = psQ.tile([128, 256], F32, tag="psQ")
                for g in range(2):
                    nc.tensor.matmul(ps[:],
                                     bds[:, g, me * 128:me * 128 + 128],
                                     w[pwT][:, g, :], start=(g == 0),
                                     stop=(g == 1))
                scopy(m1t[:, me, :, :],
                      ps.rearrange("p (a b) -> p a b", a=2))
            return m1t

        # ========== final: proj in PSUM, + bias + conv2-acc, store ==========
        def proj_pass(d, m1t):
            vb, ob, o_dram, acc = {
                "x": (vbx, "obx", out_x, acc_x),
                "y": (vby, "oby", out_y, acc_y),
            }[d]
            for tt in range(8):
                o_t = ot.tile([128, 2, 4, 128], F32, tag="o_t")
                for mo in range(2):
                    ps = psA.tile([128, 512], F32, tag="psA")
                    for ke in range(2):
                        rhs = vb[:, ke, 4 * tt + 2:4 * tt + 6, 1:129]
                        nc.tensor.matmul(ps[:], m1t[:, ke, mo, :], rhs,
                                         start=(ke == 0), stop=(ke == 1),
                                         skip_group_check=True)
                    nc.vector.scalar_tensor_tensor(
                        o_t[:, mo, :, :],
                        ps.rearrange("p (r c) -> p r c", c=128),
                        w[ob][:, mo:mo + 1],
                        acc[:, mo, 4 * tt:4 * tt + 4, :],
                        OP.add, OP.add)
                nc.sync.dma_start(
                    o_dram.ap()[:, :, tt * 512:(tt + 1) * 512],
                    o_t.rearrange("p a r c -> p a (r c)"))

        m1t_x = softmax_m1t("x")
        # the remaining conv2 blocks give the PE work to chew on while
        # the x softmax chain runs on Act/DVE
        for tt in range(4, 8):
            conv2_block(gy, acc_y, tt, False)
        m1t_y = softmax_m1t("y")
        proj_pass("x", m1t_x)
        proj_pass("y", m1t_y)

    nc.finalize()
    return nc


# ======================= host side =======================

def _prep_core_input(full, b, h0):
    """(H, W, C) rows [h0-2, h0+34) -> channel-major [128, 2, EN] f32
    (zeros outside the image)."""
    arr = np.zeros((ER, W, C), np.float32)
    r0, r1 = h0 - 2, h0 + RB + 2
    cr0, cr1 = max(r0, 0), min(r1, H)
    arr[cr0 - r0:cr1 - r0] = full[b, cr0:cr1]
    cm = arr.transpose(2, 0, 1).reshape(2, 128, EN)
    return np.ascontiguousarray(cm.transpose(1, 0, 2)).astype(ml_dtypes.bfloat16)


def _cm(v):
    return np.ascontiguousarray(v.reshape(2, 128).T.astype(np.float32))


def _lhsT(wm, nk):
    t = wm.T.reshape(nk, 128, 2, 128)
    return np.ascontiguousarray(t.transpose(1, 0, 2, 3).astype(np.float32))


def _rhsT(wm, dt=np.float32):
    t = wm.T.reshape(2, 128, wm.shape[0])
    return np.ascontiguousarray(t.transpose(1, 0, 2).astype(dt))


def _pack(parts, spec, offs, total, dtype):
    blob = np.zeros((128, total), dtype)
    for name, _ in spec:
        o, n, tail = offs[name]
        blob[:, o:o + n] = parts[name].reshape(128, n).astype(dtype)
    return blob


def kernel(_trace=False, **inputs):
    inp = {k: np.asarray(v) for k, v in inputs.items()}
    bf = ml_dtypes.bfloat16

    w2c = inp["pe_w2"].reshape(256, 9).astype(np.float32)
    dw2 = np.zeros((128, 2, 9, 128), np.float32)
    for g in range(2):
        for t in range(9):
            dw2[np.arange(128), g, t, np.arange(128)] = \
                w2c[g * 128:(g + 1) * 128, t]

    pf = {
        "pxwT": _rhsT(inp["px_w"]), "pywT": _rhsT(inp["py_w"]),
        "blk128": np.kron(np.eye(4), np.ones((32, 32))).astype(np.float32),
        "eye32r": np.tile(np.eye(32), (4, 1)).astype(np.float32),
        "eye4": np.ascontiguousarray(np.broadcast_to(
            np.tile(np.eye(32), (4, 1))[:, None, None, :],
            (128, 2, 2, 32))).astype(np.float32),
        "w1c": np.ascontiguousarray(
            inp["pe_w1"].reshape(256, 9).reshape(2, 128, 9)
            .transpose(1, 0, 2).astype(np.float32)),
        "bfx": _cm(inp["fx_b1"]), "bfy": _cm(inp["fy_b1"]),
        "bq": _cm(inp["q_b1"]), "bv": _cm(inp["v_b1"]),
        "bkx": _cm(inp["k_w1"] @ inp["fx_b2"] + inp["k_b1"]),
        "bky": _cm(inp["k_w1"] @ inp["fy_b2"] + inp["k_b1"]),
        "obx": _cm(inp["px_b"] + inp["pe_b2"]),
        "oby": _cm(inp["py_b"] + inp["pe_b2"]),
        "b1c": _cm(inp["pe_b1"]),
        "rx_exp": np.ascontiguousarray(
            np.repeat(inp["rescale_x"].reshape(2, 4), 32, axis=1).T
            .astype(np.float32)),
        "ry_exp": np.ascontiguousarray(
            np.repeat(inp["rescale_y"].reshape(2, 4), 32, axis=1).T
            .astype(np.float32)),
        "gm0": np.ones((128, 1), np.float32),
        "gm33": np.ones((128, 1), np.float32),
    }
    pb = {
        "fxw1T": _lhsT(inp["fx_w1"], 4), "fyw1T": _lhsT(inp["fy_w1"], 4),
        "qw1T": _lhsT(inp["q_w1"], 2), "vw1T": _lhsT(inp["v_w1"], 2),
        "kxw1T": _lhsT(inp["k_w1"] @ inp["fx_w2"], 2),
        "kyw1T": _lhsT(inp["k_w1"] @ inp["fy_w2"], 2),
        "vw2T": _lhsT(inp["v_w2"], 2),
        "qw2T": _rhsT(inp["q_w2"], bf), "kw2T": _rhsT(inp["k_w2"], bf),
        "dw2": dw2.astype(bf),
    }
    wf_shared = _pack(pf, SPEC_F32, OFF_F32, NF32, np.float32)
    wb_shared = _pack(pb, SPEC_B16, OFF_B16, NB16, bf)
    o0 = OFF_F32["gm0"][0]
    o33 = OFF_F32["gm33"][0]

    in_maps = []
    for r in range(8):
        b, h0 = r // 4, (r % 4) * RB
        wf = wf_shared.copy()
        wf[:, o0] = 0.0 if h0 == 0 else 1.0
        wf[:, o33] = 0.0 if h0 + RB == H else 1.0
        in_maps.append({
            "xin": _prep_core_input(inp["x_in"], b, h0),
            "yin": _prep_core_input(inp["y_in"], b, h0),
            "wf": wf,
            "wb": wb_shared,
        })

    if "nc" not in _CACHED:
        _CACHED["nc"] = _nc_build()
    res = run_bass_kernel_spmd(_CACHED["nc"], in_maps,
                               core_ids=list(range(8)), trace=_trace)
    _CACHED["last_result"] = res

    out_x = np.empty((B, H, W, C), np.float32)
    out_y = np.empty((B, H, W, C), np.float32)
    for r in range(8):
        b, h0 = r // 4, (r % 4) * RB
        for name, dst in (("out_x", out_x), ("out_y", out_y)):
            a = res.results[r][name].reshape(128, 2, RB, W)
            dst[b, h0:h0 + RB] = a.transpose(2, 3, 1, 0).reshape(RB, W, C)
    return out_x, out_y


# revision 30
# speedup vs baseline: 1.0827x; 1.0039x over previous
"""DMSA (dual-modal channel cross-attention) Trainium2 kernel — v5.

Sharding: 8 cores = 2 batches x 4 bands of 32 image rows. Each core
computes its band fully; the channel attention's per-head Gram matrices
(contraction over all n = h*w tokens, with l2-normalization folded in
via the Gram diagonal) are summed with one AllReduce per 4-core group.

Key design points (vs the 606us v2 baseline, ~1.7x faster):
- All weights packed into one bf16 + one f32r blob (4 big DMAs): the
  v2 per-tensor loads produced thousands of 8-byte DMA packets that
  serialized ~100us of dead time before the first matmul.
- v kept in SBUF (bf16 [128,2,36,130] per modality); no DRAM spill or
  re-read (-30MB DMA/core), conv taps and stage-3 matmul rhs read it
  in place.
- Whole stage-1 stream in bf16 (PE forbids mixing 32/16-bit matmul
  operands); f32r retained for the tiny softmax/proj-matrix path.
- Activation-table discipline: Prelu (not Lrelu) + Copy + Gelu all
  live in one Act table set -> 4 table loads total instead of 37.
  PSUM evictions ride the Act engine (Copy) to keep DVE free for the
  9-tap conv1; conv1 output gelu'd in place per chunk.
- Compact f32 collective payload [128,8,64] (own-block diag cols +
  cross block) instead of the full [8,128,128] Gram: 256KB vs 512KB,
  extracted straight from PSUM.
- conv2 (pos-emb second dwconv) as 9 diagonal matmuls per 512-token
  block runs during the collective window into a bf16 accumulator;
  the post-collective pass is only the fused attn+proj matmul plus a
  single DVE (psum + bias + acc) combine per tile.
"""
import numpy as np
import ml_dtypes
from contextlib import ExitStack

import concourse.bass as bass
import concourse.tile as tile
import concourse.mybir as mybir
from concourse import bacc
from concourse.bass_utils import run_bass_kernel_spmd

F32 = mybir.dt.float32
F32R = mybir.dt.float32r
BF16 = mybir.dt.bfloat16
AF = mybir.ActivationFunctionType
OP = mybir.AluOpType

B, H, W, C = 2, 128, 128, 256
HEADS, DH = 8, 32
RB = 32             # image rows per core
ER = RB + 4         # ext rows
WP = W + 2          # padded width (SBUF v grid)
EN = ER * W         # unpadded ext tokens (stage-1 grid) = 4608
NV = RB * W         # valid tokens = 4096
NT = 9              # stage-1 tiles (4 ext rows each)
LRELU_A = 0.01
# conv1 chunk g-row ranges and the stage-1 tile after which each may run
C1CHUNKS = [(0, 6, 1), (6, 12, 3), (12, 18, 4), (18, 24, 6), (24, 30, 7),
            (30, 34, None)]  # None -> last (x on DVE so CC isn't delayed)

# packed-weight layouts: (name, shape-after-partition-dim)
SPEC_F32 = [
    ("pxwT", (2, 256)), ("pywT", (2, 256)),
    ("blk128", (128,)), ("eye32r", (32,)), ("eye4", (2, 2, 32)),
    ("w1c", (2, 9)),
    ("bfx", (2,)), ("bfy", (2,)), ("bq", (2,)), ("bkx", (2,)),
    ("bky", (2,)), ("bv", (2,)), ("obx", (2,)), ("oby", (2,)),
    ("b1c", (2,)), ("rx_exp", (2,)), ("ry_exp", (2,)),
    ("gm0", (1,)), ("gm33", (1,)),
]
SPEC_B16 = [
    ("fyw1T", (4, 2, 128)), ("fxw1T", (4, 2, 128)),
    ("qw1T", (2, 2, 128)), ("kxw1T", (2, 2, 128)), ("kyw1T", (2, 2, 128)),
    ("vw1T", (2, 2, 128)), ("vw2T", (2, 2, 128)),
    ("qw2T", (2, 256)), ("kw2T", (2, 256)), ("dw2", (2, 9, 128)),
]


def _spec_offsets(spec):
    offs, o = {}, 0
    for name, tail in spec:
        n = int(np.prod(tail))
        offs[name] = (o, n, tail)
        o += n
    return offs, o


OFF_F32, NF32 = _spec_offsets(SPEC_F32)
OFF_B16, NB16 = _spec_offsets(SPEC_B16)
# bf16 blob split points so the tile-0 weights land first
WB_SPLITS = [0, 1024, 2560, 5120, NB16]

_CACHED = {}


def _nc_build():
    nc = bacc.Bacc(num_devices=8)

    xin = nc.dram_tensor("xin", [128, 2, EN], BF16, kind="ExternalInput")
    yin = nc.dram_tensor("yin", [128, 2, EN], BF16, kind="ExternalInput")
    wfd = nc.dram_tensor("wf", [128, NF32], F32R, kind="ExternalInput")
    wbd = nc.dram_tensor("wb", [128, NB16], BF16, kind="ExternalInput")

    out_x = nc.dram_tensor("out_x", [128, 2, NV], F32, kind="ExternalOutput")
    out_y = nc.dram_tensor("out_y", [128, 2, NV], F32, kind="ExternalOutput")
    cc_in = nc.dram_tensor("cc_in", [128, 8, 64], F32, kind="Internal")
    cc_out = nc.dram_tensor("cc_out", [128, 8, 64], F32, kind="Internal")

    with tile.TileContext(nc) as tc, ExitStack() as ctx:
        wp = ctx.enter_context(tc.tile_pool(name="wp", bufs=1))
        io = ctx.enter_context(tc.tile_pool(name="io", bufs=3))
        hidF = ctx.enter_context(tc.tile_pool(name="hidF", bufs=2))
        hidQ = ctx.enter_context(tc.tile_pool(name="hidQ", bufs=2))
        hidV = ctx.enter_context(tc.tile_pool(name="hidV", bufs=2))
        stk = ctx.enter_context(tc.tile_pool(name="stk", bufs=2))
        sm = ctx.enter_context(tc.tile_pool(name="sm", bufs=1))
        gb = ctx.enter_context(tc.tile_pool(name="gb", bufs=1))
        vbp = ctx.enter_context(tc.tile_pool(name="vbp", bufs=1))
        accp = ctx.enter_context(tc.tile_pool(name="accp", bufs=1))
        ot = ctx.enter_context(tc.tile_pool(name="ot", bufs=2))
        psA = ctx.enter_context(tc.tile_pool(name="psA", bufs=4, space="PSUM"))
        psQ = ctx.enter_context(tc.tile_pool(name="psQ", bufs=2, space="PSUM"))
        psG = ctx.enter_context(tc.tile_pool(name="psG", bufs=1, space="PSUM"))

        # input tile 0 DMAs go first so their packets lead the queue
        xt0 = io.tile([128, 2, 512], BF16, tag="xt")
        nc.sync.dma_start(xt0[:], xin.ap()[:, :, 0:512])
        yt0 = io.tile([128, 2, 512], BF16, tag="yt")
        nc.sync.dma_start(yt0[:], yin.ap()[:, :, 0:512])

        wf = wp.tile([128, NF32], F32R, tag="wf")
        nc.sync.dma_start(wf[:], wfd.ap())
        wb = wp.tile([128, NB16], BF16, tag="wb")
        for a, b in zip(WB_SPLITS[:-1], WB_SPLITS[1:]):
            nc.sync.dma_start(wb[:, a:b], wbd.ap()[:, a:b])

        wff = wf.bitcast(F32)

        def wview(name):
            if name in OFF_F32:
                o, n, tail = OFF_F32[name]
                v = wff[:, o:o + n]
            else:
                o, n, tail = OFF_B16[name]
                v = wb[:, o:o + n]
            if len(tail) == 2:
                v = v.rearrange("p (a b) -> p a b", a=tail[0])
            elif len(tail) == 3:
                v = v.rearrange("p (a b c) -> p a b c", a=tail[0], b=tail[1])
            return v

        def wviewr(name):
            o, n, tail = OFF_F32[name]
            v = wf[:, o:o + n]
            if len(tail) == 2:
                v = v.rearrange("p (a b) -> p a b", a=tail[0])
            elif len(tail) == 3:
                v = v.rearrange("p (a b c) -> p a b c", a=tail[0], b=tail[1])
            return v

        w = {}
        for name in ("pxwT", "pywT", "blk128"):
            w[name] = wviewr(name)
        for name in ("eye32r", "eye4", "w1c", "bfx", "bfy", "bq", "bkx", "bky",
                     "bv", "obx", "oby", "b1c", "rx_exp", "ry_exp",
                     "gm0", "gm33"):
            w[name] = wview(name)
        for name, _ in SPEC_B16:
            w[name] = wview(name)

        def scopy(out, in_):
            # Copy is resident in every Act table set -> never a table load
            nc.scalar.activation(out, in_, AF.Copy)

        # SBUF v grids (bf16, width-padded); zero the pad columns once
        vbx = vbp.tile([128, 2, ER, WP], BF16, tag="vbx")
        vby = vbp.tile([128, 2, ER, WP], BF16, tag="vby")
        for vb in (vbx, vby):
            nc.vector.memset(vb[:, :, :, 0:1], 0.0)
            nc.vector.memset(vb[:, :, :, WP - 1:WP], 0.0)

        gram0 = psG.tile([128, 512], F32, tag="gram0")
        gram1 = psG.tile([128, 512], F32, tag="gram1")
        grams = [gram0, gram1]

        gx = gb.tile([128, 2, ER - 2, WP], BF16, tag="gx")
        gy = gb.tile([128, 2, ER - 2, WP], BF16, tag="gy")
        nc.scalar.memzero(gx[:])
        nc.scalar.memzero(gy[:])
        TAPS = [(dr, dc) for dr in (-1, 0, 1) for dc in (-1, 0, 1)]

        def conv1_chunk(gbuf, vb, g0, g1):
            """9-tap conv1 for g rows [g0, g1), accumulated in gbuf (bf16,
            pre-gelu; the gelu+bias pass is batched at stage-1 end)."""
            nr = g1 - g0
            for g in range(2):
                dst = gbuf[:, g, g0:g1, 1:129]
                for i, (dr, dc) in enumerate(TAPS):
                    src = vb[:, g, g0 + 1 + dr:g0 + 1 + dr + nr,
                             1 + dc:129 + dc]
                    if i == 0:
                        nc.vector.tensor_scalar_mul(dst, src,
                                                    w["w1c"][:, g, 0:1])
                    else:
                        nc.vector.scalar_tensor_tensor(
                            dst, src, w["w1c"][:, g, i:i + 1],
                            dst, OP.mult, OP.add)

        def gelu_pass(gbuf, r0, r1):
            for g in range(2):
                nc.scalar.activation(gbuf[:, g, r0:r1, 1:129],
                                     gbuf[:, g, r0:r1, 1:129],
                                     AF.Gelu, bias=w["b1c"][:, g:g + 1])

        # ================= stage 1 =================
        vrow = 0

        def mlp1(srcs, w1T, nk, bias, tag, pool, dt, lo=0, n=512):
            """hidden = lrelu(srcs @ w1T + b); paired-bank PSUM."""
            ht = pool.tile([128, 2, 512], dt, tag=tag)
            for mh in range(2):
                ps = psA.tile([128, 512], F32, tag="psA")
                for k in range(nk):
                    src = srcs[k // 2][:, k % 2, lo:lo + n] if len(srcs) > 1 \
                        else srcs[0][:, k, lo:lo + n]
                    nc.tensor.matmul(ps[:, :n], w1T[:, k, mh, :], src,
                                     start=(k == 0), stop=(k == nk - 1))
                nc.scalar.activation(ht[:, mh, :n], ps[:, :n], AF.Prelu,
                                     bias=bias[:, mh:mh + 1], alpha=LRELU_A)
            return ht

        for t in range(NT):
            if t == 0:
                xt, yt = xt0, yt0
            else:
                xt = io.tile([128, 2, 512], BF16, tag="xt")
                nc.sync.dma_start(xt[:], xin.ap()[:, :, t * 512:(t + 1) * 512])
                yt = io.tile([128, 2, 512], BF16, tag="yt")
                nc.sync.dma_start(yt[:], yin.ap()[:, :, t * 512:(t + 1) * 512])

            # valid-row window within this tile
            e0, e1 = max(2, 4 * t), min(ER - 2, 4 * t + 4)
            lo, n = (e0 - 4 * t) * 128, (e1 - e0) * 128

            fhy = mlp1([xt, yt], w["fyw1T"], 4, w["bfy"], "fhy", hidF, BF16,
                       lo, n)
            fhx = mlp1([xt, yt], w["fxw1T"], 4, w["bfx"], "fhx", hidF, BF16,
                       lo, n)
            vhx = mlp1([xt], w["vw1T"], 2, w["bv"], "vhx", hidV, BF16)
            vhy = mlp1([yt], w["vw1T"], 2, w["bv"], "vhy", hidV, BF16)
            qhx = mlp1([xt], w["qw1T"], 2, w["bq"], "qhx", hidQ, BF16, lo, n)
            qhy = mlp1([yt], w["qw1T"], 2, w["bq"], "qhy", hidQ, BF16, lo, n)
            khy = mlp1([fhy], w["kyw1T"], 2, w["bky"], "khy", hidQ, BF16,
                       0, n)
            khx = mlp1([fhx], w["kxw1T"], 2, w["bkx"], "khx", hidQ, BF16,
                       0, n)

            # v = vhid @ vw2T (ext tokens), evict bf16 into the SBUF grid
            for vh, vb in ((vhx, vbx), (vhy, vby)):
                for mh in range(2):
                    ps = psA.tile([128, 512], F32, tag="psA")
                    for k in range(2):
                        nc.tensor.matmul(ps[:], w["vw2T"][:, k, mh, :],
                                         vh[:, k, :], start=(k == 0),
                                         stop=(k == 1))
                    scopy(vb[:, mh, 4 * t:4 * t + 4, 1:129],
                          ps.rearrange("p (r c) -> p r c", c=128))

            # token-major QK L2 + Gram per valid image row; two stacks
            # share one PSUM tile and one eviction
            for e in range(e0, e1):
                off = (e - e0) * 128
                st = stk.tile([128, HEADS, 4, DH], BF16, tag="st")
                for pair, grp in enumerate((((khy, "kw2T"), (qhx, "qw2T")),
                                            ((khx, "kw2T"), (qhy, "qw2T")))):
                    ps = psQ.tile([128, 2, 256], F32, tag="psQ")
                    for s2, (hh, w2T) in enumerate(grp):
                        for k in range(2):
                            nc.tensor.matmul(ps[:, s2, :],
                                             hh[:, k, off:off + 128],
                                             w[w2T][:, k, :], start=(k == 0),
                                             stop=(k == 1))
                    scopy(st[:, :, 2 * pair:2 * pair + 2, :],
                          ps.rearrange("p s (h d) -> p h s d", h=HEADS))
                for h in range(HEADS):
                    nc.tensor.matmul(
                        grams[h // 4][:, (h % 4) * 128:(h % 4) * 128 + 128],
                        st[:, h], st[:, h],
                        start=(vrow == 0), stop=(vrow == RB - 1),
                        skip_group_check=True)
                vrow += 1

            # interleaved conv1 chunks with in-place gelu per chunk
            for g0, g1, after in C1CHUNKS:
                if after == t:
                    for gbuf, vb in ((gx, vbx), (gy, vby)):
                        conv1_chunk(gbuf, vb, g0, g1)
                        gelu_pass(gbuf, g0, g1)
                        if g0 == 0:
                            nc.vector.tensor_scalar_mul(
                                gbuf[:, :, 0, :], gbuf[:, :, 0, :],
                                w["gm0"][:])

        # ============ compact Gram payload -> AllReduce ============
        # csb [128(stack: ky|qx|kx|qy x32), head, 64]:
        #   cols 0:32  = own-block (diag blocks, for the l2 norms)
        #   cols 32:64 = cross block (B1 = ky^T qx at p 0:32,
        #                             B2 = kx^T qy at p 64:96)
        csb = sm.tile([128, 8, 64], F32, tag="csb")
        nc.vector.memset(csb[:], 0.0)
        for g in range(2):
            grv = grams[g].rearrange("p (h c) -> p h c", h=4)
            for pr in range(4):
                nc.vector.tensor_copy(
                    csb[pr * 32:(pr + 1) * 32, 4 * g:4 * g + 4, 0:32],
                    grv[pr * 32:(pr + 1) * 32, :, pr * 32:pr * 32 + 32])
            nc.vector.tensor_copy(csb[0:32, 4 * g:4 * g + 4, 32:64],
                                  grv[0:32, :, 32:64])
            nc.vector.tensor_copy(csb[64:96, 4 * g:4 * g + 4, 32:64],
                                  grv[64:96, :, 96:128])
        nc.sync.dma_start(cc_in.ap(), csb[:])
        nc.gpsimd.collective_compute(
            "AllReduce", OP.add,
            ins=[cc_in.ap()], outs=[cc_out.ap()],
            replica_groups=[[0, 1, 2, 3], [4, 5, 6, 7]])

        # last conv1 chunk + its gelu + boundary mask overlap the CC
        for g0, g1, after in C1CHUNKS:
            if after is None:
                for gbuf, vb in ((gx, vbx), (gy, vby)):
                    conv1_chunk(gbuf, vb, g0, g1)
                    gelu_pass(gbuf, g0, g1)
                    nc.vector.tensor_scalar_mul(
                        gbuf[:, :, ER - 3, :], gbuf[:, :, ER - 3, :],
                        w["gm33"][:])


        # ====== conv2 (pos-emb second dwconv) during the collective ======
        # 9 diagonal matmuls per 512-token block into PSUM, evicted to a
        # bf16 accumulator; the post-collective pass only adds the proj.
        acc_x = accp.tile([128, 2, RB, 128], BF16, tag="acc_x")
        acc_y = accp.tile([128, 2, RB, 128], BF16, tag="acc_y")
        accs = {"x": acc_x, "y": acc_y}

        def conv2_block(gbuf, acc, tt, on_dve):
            for mo in range(2):
                ps = psA.tile([128, 512], F32, tag="psA")
                for i in range(9):
                    dr, dc = TAPS[i]
                    src = gbuf[:, mo, 4 * tt + 1 + dr:4 * tt + 5 + dr,
                               1 + dc:129 + dc]
                    nc.tensor.matmul(ps[:], w["dw2"][:, mo, i, :],
                                     src, start=(i == 0), stop=(i == 8),
                                     skip_group_check=True)
                dst = acc[:, mo, 4 * tt:4 * tt + 4, :]
                srcv = ps.rearrange("p (r c) -> p r c", c=128)
                if on_dve:
                    nc.vector.tensor_copy(dst, srcv)
                else:
                    scopy(dst, srcv)

        for tt in range(7):
            conv2_block(gx, acc_x, tt, False)
            if tt < 4:
                conv2_block(gy, acc_y, tt, False)
        conv2_block(gx, acc_x, 7, False)

        # ========== softmax + BD + fused proj matrices ==========
        # layouts from cc_out [128(stack), 8, 64] f32:
        #   x: cross at p 0:32, own k at p 0:32, own q at p 32:64
        #   y: cross at p 64:96, own k at p 64:96, own q at p 96:128
        PRE = {"x": (0, 0), "y": (64, 64)}
        s_ts, dbs = {}, {}
        for d, (pc, pk) in PRE.items():
            s_t = sm.tile([128, 2, DH], F32, tag=f"s_t{d}")
            nc.sync.dma_start(
                s_t[:],
                cc_out.ap()[pc:pc + 32, :, 32:64]
                .rearrange("d (g j) e -> j d g e", g=2))
            db = sm.tile([128, 2, 2, DH], F32, tag=f"db{d}")
            for jj in range(2):
                nc.sync.dma_start(
                    db[:, :, jj, :],
                    cc_out.ap()[pk + 32 * jj:pk + 32 * jj + 32, :, 0:32]
                    .rearrange("d (g j) e -> j d g e", g=2))
            s_ts[d], dbs[d] = s_t, db

        def softmax_m1t(d):
            rexp = "rx_exp" if d == "x" else "ry_exp"
            pwT = "pxwT" if d == "x" else "pywT"
            s_t, db = s_ts[d], dbs[d]
            dbf = sm.tile([128, 2, 2, DH], F32, tag="dbf")
            nc.vector.tensor_tensor(dbf[:], db[:], w["eye4"][:], OP.mult)
            nkq = sm.tile([128, 2, 2], F32, tag="nkq")
            nc.vector.tensor_reduce(nkq[:], dbf[:], mybir.AxisListType.X,
                                    OP.add)
            inv = sm.tile([128, 2, 2], F32, tag="inv")
            nc.scalar.sqrt(inv[:], nkq[:])
            nc.vector.tensor_scalar_max(inv[:], inv[:], 1e-12)
            nc.vector.reciprocal(inv[:], inv[:])
            ks = sm.tile([128, 2], F32, tag="ks")
            nc.vector.tensor_tensor(ks[:], inv[:, :, 0], w[rexp][:], OP.mult)
            qs = sm.tile([128, 2, DH], F32, tag="qs")
            for g in range(2):
                eis = sm.tile([128, DH], F32, tag="eis")
                nc.vector.tensor_scalar_mul(eis[:], w["eye32r"][:],
                                            inv[:, g, 1:2])
                ei = sm.tile([128, DH], F32R, tag="ei")
                nc.vector.tensor_copy(ei[:], eis[:])
                pq_ = psQ.tile([128, DH], F32, tag="psQ")
                nc.tensor.matmul(pq_[:], w["blk128"][:], ei[:],
                                 start=True, stop=True)
                scopy(qs[:, g, :], pq_[:])
            # logits are cosine similarities (|lg| <= rescale), so exp is
            # safe without the max-subtraction pass
            lg = sm.tile([128, 2, DH], F32, tag="lg")
            for g in range(2):
                nc.vector.scalar_tensor_tensor(lg[:, g, :], s_t[:, g, :],
                                               ks[:, g:g + 1], qs[:, g, :],
                                               OP.mult, OP.mult)
            pe_ = sm.tile([128, 2, DH], F32, tag="pe_")
            ssum = sm.tile([128, 2], F32, tag="ssum")
            for g in range(2):
                nc.scalar.activation(pe_[:, g, :], lg[:, g, :], AF.Exp,
                                     accum_out=ssum[:, g:g + 1])
            nc.vector.reciprocal(ssum[:], ssum[:])
            at = sm.tile([128, 2, DH], F32, tag="at")
            for g in range(2):
                nc.vector.tensor_scalar_mul(at[:, g, :], pe_[:, g, :],
                                            ssum[:, g:g + 1])
            bds = sm.tile([128, 2, 256], F32R, tag="bds")
            nc.vector.memset(bds.bitcast(F32)[:], 0.0)
            for g in range(2):
                for j in range(4):
                    h = 4 * g + j
                    nc.vector.tensor_copy(
                        bds[j * DH:(j + 1) * DH, g, h * DH:(h + 1) * DH],
                        at[j * DH:(j + 1) * DH, g, :])
            m1t = sm.tile([128, 2, 2, 128], BF16, tag=f"m1t_{d}")
            for me in range(2):
                ps = psQ.tile([128, 256], F32, tag="psQ")
                for g in range(2):
                    nc.tensor.matmul(ps[:],
                                     bds[:, g, me * 128:me * 128 + 128],
                                     w[pwT][:, g, :], start=(g == 0),
                                     stop=(g == 1))
                scopy(m1t[:, me, :, :],
                      ps.rearrange("p (a b) -> p a b", a=2))
            return m1t

        # ========== final: proj in PSUM, + bias + conv2-acc, store ==========
        def proj_pass(d, m1t):
            vb, ob, o_dram, acc = {
                "x": (vbx, "obx", out_x, acc_x),
                "y": (vby, "oby", out_y, acc_y),
            }[d]
            for tt in range(8):
                o_t = ot.tile([128, 2, 4, 128], F32, tag="o_t")
                for mo in range(2):
                    ps = psA.tile([128, 512], F32, tag="psA")
                    for ke in range(2):
                        rhs = vb[:, ke, 4 * tt + 2:4 * tt + 6, 1:129]
                        nc.tensor.matmul(ps[:], m1t[:, ke, mo, :], rhs,
                                         start=(ke == 0), stop=(ke == 1),
                                         skip_group_check=True)
                    nc.vector.scalar_tensor_tensor(
                        o_t[:, mo, :, :],
                        ps.rearrange("p (r c) -> p r c", c=128),
                        w[ob][:, mo:mo + 1],
                        acc[:, mo, 4 * tt:4 * tt + 4, :],
                        OP.add, OP.add)
                nc.sync.dma_start(
                    o_dram.ap()[:, :, tt * 512:(tt + 1) * 512],
                    o_t.rearrange("p a r c -> p a (r c)"))

        m1t_x = softmax_m1t("x")
        # the remaining conv2 blocks give the PE work to chew on while
        # the x softmax chain runs on Act/DVE
        for tt in range(4, 8):
            conv2_block(gy, acc_y, tt, False)
        m1t_y = softmax_m1t("y")
        proj_pass("x", m1t_x)
        proj_pass("y", m1t_y)

    nc.finalize()
    return nc


# ======================= host side =======================

def _prep_core_input(full, b, h0):
    """(H, W, C) rows [h0-2, h0+34) -> channel-major [128, 2, EN] f32
    (zeros outside the image)."""
    arr = np.zeros((ER, W, C), np.float32)
    r0, r1 = h0 - 2, h0 + RB + 2
    cr0, cr1 = max(r0, 0), min(r1, H)
    arr[cr0 - r0:cr1 - r0] = full[b, cr0:cr1]
    cm = arr.transpose(2, 0, 1).reshape(2, 128, EN)
    return np.ascontiguousarray(cm.transpose(1, 0, 2)).astype(ml_dtypes.bfloat16)


def _cm(v):
    return np.ascontiguousarray(v.reshape(2, 128).T.astype(np.float32))


def _lhsT(wm, nk):
    t = wm.T.reshape(nk, 128, 2, 128)
    return np.ascontiguousarray(t.transpose(1, 0, 2, 3).astype(np.float32))


def _rhsT(wm, dt=np.float32):
    t = wm.T.reshape(2, 128, wm.shape[0])
    return np.ascontiguousarray(t.transpose(1, 0, 2).astype(dt))


def _pack(parts, spec, offs, total, dtype):
    blob = np.zeros((128, total), dtype)
    for name, _ in spec:
        o, n, tail = offs[name]
        blob[:, o:o + n] = parts[name].reshape(128, n).astype(dtype)
    return blob


def kernel(_trace=False, **inputs):
    inp = {k: np.asarray(v) for k, v in inputs.items()}
    bf = ml_dtypes.bfloat16

    w2c = inp["pe_w2"].reshape(256, 9).astype(np.float32)
    dw2 = np.zeros((128, 2, 9, 128), np.float32)
    for g in range(2):
        for t in range(9):
            dw2[np.arange(128), g, t, np.arange(128)] = \
                w2c[g * 128:(g + 1) * 128, t]

    pf = {
        "pxwT": _rhsT(inp["px_w"]), "pywT": _rhsT(inp["py_w"]),
        "blk128": np.kron(np.eye(4), np.ones((32, 32))).astype(np.float32),
        "eye32r": np.tile(np.eye(32), (4, 1)).astype(np.float32),
        "eye4": np.ascontiguousarray(np.broadcast_to(
            np.tile(np.eye(32), (4, 1))[:, None, None, :],
            (128, 2, 2, 32))).astype(np.float32),
        "w1c": np.ascontiguousarray(
            inp["pe_w1"].reshape(256, 9).reshape(2, 128, 9)
            .transpose(1, 0, 2).astype(np.float32)),
        "bfx": _cm(inp["fx_b1"]), "bfy": _cm(inp["fy_b1"]),
        "bq": _cm(inp["q_b1"]), "bv": _cm(inp["v_b1"]),
        "bkx": _cm(inp["k_w1"] @ inp["fx_b2"] + inp["k_b1"]),
        "bky": _cm(inp["k_w1"] @ inp["fy_b2"] + inp["k_b1"]),
        "obx": _cm(inp["px_b"] + inp["pe_b2"]),
        "oby": _cm(inp["py_b"] + inp["pe_b2"]),
        "b1c": _cm(inp["pe_b1"]),
        "rx_exp": np.ascontiguousarray(
            np.repeat(inp["rescale_x"].reshape(2, 4), 32, axis=1).T
            .astype(np.float32)),
        "ry_exp": np.ascontiguousarray(
            np.repeat(inp["rescale_y"].reshape(2, 4), 32, axis=1).T
            .astype(np.float32)),
        "gm0": np.ones((128, 1), np.float32),
        "gm33": np.ones((128, 1), np.float32),
    }
    pb = {
        "fxw1T": _lhsT(inp["fx_w1"], 4), "fyw1T": _lhsT(inp["fy_w1"], 4),
        "qw1T": _lhsT(inp["q_w1"], 2), "vw1T": _lhsT(inp["v_w1"], 2),
        "kxw1T": _lhsT(inp["k_w1"] @ inp["fx_w2"], 2),
        "kyw1T": _lhsT(inp["k_w1"] @ inp["fy_w2"], 2),
        "vw2T": _lhsT(inp["v_w2"], 2),
        "qw2T": _rhsT(inp["q_w2"], bf), "kw2T": _rhsT(inp["k_w2"], bf),
        "dw2": dw2.astype(bf),
    }
    wf_shared = _pack(pf, SPEC_F32, OFF_F32, NF32, np.float32)
    wb_shared = _pack(pb, SPEC_B16, OFF_B16, NB16, bf)
    o0 = OFF_F32["gm0"][0]
    o33 = OFF_F32["gm33"][0]

    in_maps = []
    for r in range(8):
        b, h0 = r // 4, (r % 4) * RB
        wf = wf_shared.copy()
        wf[:, o0] = 0.0 if h0 == 0 else 1.0
        wf[:, o33] = 0.0 if h0 + RB == H else 1.0
        in_maps.append({
            "xin": _prep_core_input(inp["x_in"], b, h0),
            "yin": _prep_core_input(inp["y_in"], b, h0),
            "wf": wf,
            "wb": wb_shared,
        })

    if "nc" not in _CACHED:
        _CACHED["nc"] = _nc_build()
    res = run_bass_kernel_spmd(_CACHED["nc"], in_maps,
                               core_ids=list(range(8)), trace=_trace)
    _CACHED["last_result"] = res

    out_x = np.empty((B, H, W, C), np.float32)
    out_y = np.empty((B, H, W, C), np.float32)
    for r in range(8):
        b, h0 = r // 4, (r % 4) * RB
        for name, dst in (("out_x", out_x), ("out_y", out_y)):
            a = res.results[r][name].reshape(128, 2, RB, W)
            dst[b, h0:h0 + RB] = a.transpose(2, 3, 1, 0).reshape(RB, W, C)
    return out_x, out_y
